# revision 1
# baseline (speedup 1.0000x reference)
"""SlotAttention kernel for 8 Trainium2 NeuronCores.

Sharding: data-parallel over batch (B=32 -> 4 per core), all params
(<100KB) replicated on every core, per the problem's sharding hint.
Each core runs the full 3-iteration slot-attention loop on its batch
shard; outputs are gathered on host. No cross-core communication is
needed because every tensor contraction is within a single batch
element.
"""
import numpy as np
import jax
import jax.numpy as jnp

B, N, K = 32, 16384, 11
D_IN, D_C, D_S, HID = 64, 64, 64, 128
N_ITER = 3
EPS_LN = 1e-5
NCORES = 8

_PARAM_NAMES = ('mu', 'sigma', 'Wq', 'bq', 'Wk', 'bk', 'Wv', 'bv',
                'W_ih', 'b_ih', 'W_hh', 'b_hh', 'W1', 'b1', 'W2', 'b2')


def _ln(x):
    m = jnp.mean(x, axis=-1, keepdims=True)
    v = jnp.mean((x - m) ** 2, axis=-1, keepdims=True)
    return (x - m) / jnp.sqrt(v + EPS_LN)


def _shard_fn(inputs, slot_noise, mu, sigma, Wq, bq, Wk, bk, Wv, bv,
              W_ih, b_ih, W_hh, b_hh, W1, b1, W2, b2):
    # inputs: [b, N, D_IN], slot_noise: [b, K, D_C] for this core's shard
    b = inputs.shape[0]
    slots = mu + sigma * slot_noise                      # [b,K,Ds]
    x = _ln(inputs)
    k = x @ Wk.T + bk                                    # [b,N,Dc]
    v = x @ Wv.T + bv                                    # [b,N,Dc]
    scale = 1.0 / np.sqrt(D_C)
    attn_t = None
    for _ in range(N_ITER):
        q = _ln(slots) @ Wq.T + bq                       # [b,K,Dc]
        scores = jnp.einsum('bnd,bkd->bnk', k, q,
                            precision=jax.lax.Precision.HIGHEST) * scale
        attn = jax.nn.softmax(scores, axis=2) + 1e-8     # softmax over slots
        attn = attn / jnp.sum(attn, axis=1, keepdims=True)  # renorm over inputs
        attn_t = jnp.transpose(attn, (0, 2, 1))          # [b,K,N]
        updates = jnp.einsum('bnd,bkn->bkd', v, attn_t,
                             precision=jax.lax.Precision.HIGHEST)
        xg = updates.reshape(-1, D_C) @ W_ih.T + b_ih
        hg = slots.reshape(-1, D_S) @ W_hh.T + b_hh
        xr, xz, xn = jnp.split(xg, 3, axis=-1)
        hr, hz, hn = jnp.split(hg, 3, axis=-1)
        r = jax.nn.sigmoid(xr + hr)
        z = jax.nn.sigmoid(xz + hz)
        n = jnp.tanh(xn + r * hn)
        h = (1.0 - z) * n + z * slots.reshape(-1, D_S)
        slots = _ln(h.reshape(b, K, D_S))
        slots = slots + (jax.nn.relu(slots @ W1.T + b1) @ W2.T + b2)
    masks = attn_t.reshape(b, K, 128, 128)
    return slots, masks


_pmapped = None


def _get_pmapped():
    global _pmapped
    if _pmapped is None:
        _pmapped = jax.pmap(
            _shard_fn,
            in_axes=(0, 0) + (None,) * len(_PARAM_NAMES),
            devices=jax.devices()[:NCORES],
        )
    return _pmapped


def kernel(**inputs):
    x = np.ascontiguousarray(inputs['inputs'], dtype=np.float32)
    sn = np.ascontiguousarray(inputs['slot_noise'], dtype=np.float32)
    per = B // NCORES
    x_sh = x.reshape(NCORES, per, N, D_IN)
    sn_sh = sn.reshape(NCORES, per, K, D_C)
    params = [np.asarray(inputs[p], dtype=np.float32) for p in _PARAM_NAMES]
    slots, masks = _get_pmapped()(x_sh, sn_sh, *params)
    slots = np.asarray(slots, dtype=np.float32).reshape(B, K, D_S)
    masks = np.asarray(masks, dtype=np.float32).reshape(B, K, 128, 128)
    return slots, masks


# revision 3
# speedup vs baseline: 37.8890x; 37.8890x over previous
"""SlotAttention kernel for 8 Trainium2 NeuronCores.

Sharding: data-parallel over batch (B=32 -> 4 per core), all params
(<100KB) replicated on every core, per the problem's sharding hint.
Each core runs the full 3-iteration slot-attention loop on its batch
shard; outputs are gathered on host. No cross-core communication is
needed because every tensor contraction is within a single batch
element.
"""
import numpy as np
import jax
import jax.numpy as jnp

B, N, K = 32, 16384, 11
D_IN, D_C, D_S, HID = 64, 64, 64, 128
N_ITER = 3
EPS_LN = 1e-5
NCORES = 8

_PARAM_NAMES = ('mu', 'sigma', 'Wq', 'bq', 'Wk', 'bk', 'Wv', 'bv',
                'W_ih', 'b_ih', 'W_hh', 'b_hh', 'W1', 'b1', 'W2', 'b2')


def _ln(x):
    m = jnp.mean(x, axis=-1, keepdims=True)
    v = jnp.mean((x - m) ** 2, axis=-1, keepdims=True)
    return (x - m) / jnp.sqrt(v + EPS_LN)


def _shard_fn(inputs, slot_noise, mu, sigma, Wq, bq, Wk, bk, Wv, bv,
              W_ih, b_ih, W_hh, b_hh, W1, b1, W2, b2):
    # inputs: [b, N, D_IN], slot_noise: [b, K, D_C] for this core's shard
    b = inputs.shape[0]
    slots = mu + sigma * slot_noise                      # [b,K,Ds]
    x = _ln(inputs)                                      # [b,N,D]
    scale = 1.0 / np.sqrt(D_C)
    attn_t = None
    for _ in range(N_ITER):
        q = _ln(slots) @ Wq.T + bq                       # [b,K,Dc]
        # scores.T[k,n] = scale*(k_proj[n]@q[k]) with k_proj = x@Wk.T + bk
        #              = scale*(x[n]@(Wk.T q[k]) + bk@q[k])
        qeff = (q @ Wk) * scale                          # [b,K,D]
        c0 = scale * (q @ bk)                            # [b,K]
        scores_t = jnp.einsum('bkd,bnd->bkn', qeff, x) + c0[:, :, None]
        attn_t = jax.nn.softmax(scores_t, axis=1) + 1e-8   # softmax over slots
        attn_t = attn_t / jnp.sum(attn_t, axis=2, keepdims=True)  # renorm over n
        # updates[k] = sum_n attn_t[k,n] * (x[n]@Wv.T + bv)
        #            = (attn_t @ x) @ Wv.T + bv   (rows of attn_t sum to 1)
        updates = jnp.einsum('bkn,bnd->bkd', attn_t, x) @ Wv.T + bv
        xg = updates.reshape(-1, D_C) @ W_ih.T + b_ih
        hg = slots.reshape(-1, D_S) @ W_hh.T + b_hh
        xr, xz, xn = jnp.split(xg, 3, axis=-1)
        hr, hz, hn = jnp.split(hg, 3, axis=-1)
        r = jax.nn.sigmoid(xr + hr)
        z = jax.nn.sigmoid(xz + hz)
        n = jnp.tanh(xn + r * hn)
        h = (1.0 - z) * n + z * slots.reshape(-1, D_S)
        slots = _ln(h.reshape(b, K, D_S))
        slots = slots + (jax.nn.relu(slots @ W1.T + b1) @ W2.T + b2)
    masks = attn_t.reshape(b, K, 128, 128)  # attn_t is already [b,K,N]
    return slots, masks


_pmapped = None


def _get_pmapped():
    global _pmapped
    if _pmapped is None:
        _pmapped = jax.pmap(
            _shard_fn,
            in_axes=(0, 0) + (None,) * len(_PARAM_NAMES),
            devices=jax.devices()[:NCORES],
        )
    return _pmapped


def kernel(**inputs):
    x = np.ascontiguousarray(inputs['inputs'], dtype=np.float32)
    sn = np.ascontiguousarray(inputs['slot_noise'], dtype=np.float32)
    per = B // NCORES
    x_sh = x.reshape(NCORES, per, N, D_IN)
    sn_sh = sn.reshape(NCORES, per, K, D_C)
    params = [np.asarray(inputs[p], dtype=np.float32) for p in _PARAM_NAMES]
    slots, masks = _get_pmapped()(x_sh, sn_sh, *params)
    slots = np.asarray(slots, dtype=np.float32).reshape(B, K, D_S)
    masks = np.asarray(masks, dtype=np.float32).reshape(B, K, 128, 128)
    return slots, masks
